# revision 24
# baseline (speedup 1.0000x reference)
"""Causal self-attention + cross-attention Trainium2 kernel (8 NeuronCores).

Sharding: head-parallel. 16 heads x 2 batches = 32 (b,h) pairs; core c owns
heads {2c, 2c+1} for both batches (its 128 channels of C=1024). Projections
are column-sliced per core; attention runs fully local per head; the output
projection is row-sliced and the 8 partial [NT, C] fp32 outputs are summed
on the host (no device collectives).

v2: single software-pipelined emission. ScalarE runs ONLY the exp stream
(the kernel's critical resource); projections, the output projection and
V-layout work are interleaved into the PE queue as filler between the
per-k-tile score/AV steps so PE and ScalarE stay busy concurrently from
~8us onward. PSUM->SBUF evictions run on GpSimd/DVE, output tiles DMA
straight from PSUM, V is projected token-major directly (no transposes),
and the softmax epilogue runs in fp16 with a 1/64-scaled ones column.
"""
import sys

sys.path.insert(0, "/opt/trn_rl_repo")

import numpy as np

import concourse.bass as bass
import concourse.tile as tile
from concourse import bacc, mybir
from concourse.bass_utils import run_bass_kernel_spmd

dt = mybir.dt

B, T, TC, C, CC, H, D = 2, 2048, 512, 1024, 512, 16, 64
NCORES = 8
CPC = 128          # channels per core = 2 heads * 64
NT = B * T         # 4096 tokens (batch-major)
NTC = B * TC       # 1024 cross tokens
KT_X = C // 128    # 8 contraction tiles over C
KT_C = CC // 128   # 4 contraction tiles over CC
NCH = NT // 512    # 8 token chunks (b0: 0-3, b1: 4-7)
NCHC = NTC // 512  # 2 cross chunks (b0, b1)
QC_PER_B = T // 512
KT_PER_B = T // 128
ALPHA = 1.0 / 64   # ones-column value; denominators come out scaled by ALPHA


def _build(zero_bias=False):
    f32, f16 = dt.float32, dt.float16
    nc = bacc.Bacc("TRN2", target_bir_lowering=False, debug=False,
                   enable_asserts=True, num_devices=NCORES)

    xTd = nc.dram_tensor("xT", [NCH, 128, KT_X, 512], f16, kind="ExternalInput").ap()
    cTd = nc.dram_tensor("cT", [NCHC, 128, KT_C, 512], f16, kind="ExternalInput").ap()
    wqd = nc.dram_tensor("wq", [128, KT_X, CPC], f16, kind="ExternalInput").ap()
    wkd = nc.dram_tensor("wk", [128, KT_X, CPC], f16, kind="ExternalInput").ap()
    wvd = nc.dram_tensor("wv", [128, KT_X, CPC], f16, kind="ExternalInput").ap()
    wcqd = nc.dram_tensor("wcq", [128, KT_X, CPC], f16, kind="ExternalInput").ap()
    wckd = nc.dram_tensor("wck", [128, KT_C, CPC], f16, kind="ExternalInput").ap()
    wcvd = nc.dram_tensor("wcv", [128, KT_C, CPC], f16, kind="ExternalInput").ap()
    wpd = nc.dram_tensor("wp", [CPC, C], f16, kind="ExternalInput").ap()
    bias6d = nc.dram_tensor("bias6", [CPC, 6], f32, kind="ExternalInput").ap()
    maskd = nc.dram_tensor("mask", [128, 128], f16, kind="ExternalInput").ap()
    outd = nc.dram_tensor("out", [NT, C], f16, kind="ExternalOutput").ap()

    Exp = mybir.ActivationFunctionType.Exp
    Mult = mybir.AluOpType.mult
    SCALE = 0.125  # 1/sqrt(D)

    with tile.TileContext(nc) as tc:
        from contextlib import ExitStack
        with ExitStack() as es:
            persist = es.enter_context(tc.tile_pool(name="persist", bufs=1))
            qT = persist.tile([128, NT], f16, tag="qT")
            kT = persist.tile([128, NT], f16, tag="kT")
            qcT = persist.tile([128, NT], f16, tag="qcT")
            kcT = persist.tile([128, NTC], f16, tag="kcT")
            vn = persist.tile([128, (NT // 128) * 256], f16, tag="vn")
            vcn = persist.tile([128, (NTC // 128) * 256], f16, tag="vcn")
            yT2 = persist.tile([128, NT], f16, tag="yT2")
            wp_t = persist.tile([128, C], f16, tag="wp")
            bias_t = persist.tile([128, 6], f32, tag="bias")
            mask_t = persist.tile([128, 128], f16, tag="mask")
            xall = persist.tile([128, KT_X, NT], f16, tag="xall")
            call = persist.tile([128, KT_C, NTC], f16, tag="call")
            wq_t = persist.tile([128, KT_X, CPC], f16, tag="wqw")
            wk_t = persist.tile([128, KT_X, CPC], f16, tag="wkw")
            wv_t = persist.tile([128, KT_X, CPC], f16, tag="wvw")
            wcq_t = persist.tile([128, KT_X, CPC], f16, tag="wcqw")
            wck_t = persist.tile([128, KT_C, CPC], f16, tag="wckw")
            wcv_t = persist.tile([128, KT_C, CPC], f16, tag="wcvw")

            expool = es.enter_context(tc.tile_pool(name="expool", bufs=6))
            bpool = es.enter_context(tc.tile_pool(name="bpool", bufs=3))
            ypool = es.enter_context(tc.tile_pool(name="ypool", bufs=3))
            cpool = es.enter_context(tc.tile_pool(name="cpool", bufs=3))
            vtpool = es.enter_context(tc.tile_pool(name="vtpool", bufs=2))
            rdpool = es.enter_context(tc.tile_pool(name="rdpool", bufs=3, space="DRAM"))
            stps = es.enter_context(tc.tile_pool(name="stps", bufs=2, space="PSUM"))
            yh0ps = es.enter_context(tc.tile_pool(name="yh0ps", bufs=1, space="PSUM"))
            yh1ps = es.enter_context(tc.tile_pool(name="yh1ps", bufs=1, space="PSUM"))
            auxps = es.enter_context(tc.tile_pool(name="auxps", bufs=2, space="PSUM"))

            # ---------------- input DMAs (SP queue, critical first) ----
            for wdram, wtile in ((wckd, wck_t), (wcvd, wcv_t)):
                nc.sync.dma_start(out=wtile[:], in_=wdram[:])
            nc.sync.dma_start(out=mask_t[:], in_=maskd[:])
            nc.sync.dma_start(out=call[:, :, 0:512], in_=cTd[0])
            nc.sync.dma_start(out=wcq_t[:], in_=wcqd[:])
            nc.sync.dma_start(out=xall[:, :, 0:512], in_=xTd[0])
            for wdram, wtile in ((wkd, wk_t), (wvd, wv_t), (wqd, wq_t)):
                nc.sync.dma_start(out=wtile[:], in_=wdram[:])
            nc.sync.dma_start(out=bias_t[:], in_=bias6d[:])
            for ch in range(1, 4):
                nc.sync.dma_start(out=xall[:, :, ch * 512:(ch + 1) * 512],
                                  in_=xTd[ch])
            nc.sync.dma_start(out=wp_t[:], in_=wpd[:])
            nc.sync.dma_start(out=call[:, :, 512:1024], in_=cTd[1])
            for ch in range(4, 8):
                nc.sync.dma_start(out=xall[:, :, ch * 512:(ch + 1) * 512],
                                  in_=xTd[ch])

            # zero only the pad columns (yh pad rows are never read, but the
            # sim's nnan checks see matmul outputs; keep pads defined)
            vn_h = vn[:].rearrange("p (t h c) -> p t h c", h=2, c=128)
            nc.gpsimd.memset(vn_h[:, :, :, 1:64], 0.0)
            nc.gpsimd.memset(vn_h[:, :, :, 0:1], ALPHA)
            vcn_h = vcn[:].rearrange("p (t h c) -> p t h c", h=2, c=128)
            nc.gpsimd.memset(vcn_h[:, :, :, 1:64], 0.0)
            nc.gpsimd.memset(vcn_h[:, :, :, 0:1], ALPHA)

            # ---------------- filler machinery ----------------
            FILL = []            # list of (ns_est, fn, label_or_None)
            pending = set()

            def run_item(item):
                _, fn, label = item
                fn()
                if label is not None:
                    pending.discard(label)

            def pump(budget_ns):
                while budget_ns > 0 and FILL:
                    item = FILL.pop(0)
                    run_item(item)
                    budget_ns -= item[0]

            def require(label):
                while label in pending:
                    run_item(FILL.pop(0))

            def evict(dst, ps, bcol, eng):
                if zero_bias:
                    eng.tensor_copy(dst, ps)
                else:
                    eng.tensor_scalar_add(dst, ps, bias_t[:, bcol:bcol + 1])

            # channel-major projection group: dst[:, ch*512:+512] = W^T x_chunk
            def proj_items(wtile, nkt, xsrc, ch, dst, bcol, label):
                items = []
                state = {}
                for kt in range(nkt):
                    def f(kt=kt, state=state):
                        if kt == 0:
                            state['ps'] = auxps.tile([128, 512], f32, tag="aux", name="auxtile")
                        nc.tensor.matmul(state['ps'][:], wtile[:, kt, :],
                                         xsrc[:, kt, ch * 512:(ch + 1) * 512],
                                         start=(kt == 0), stop=(kt == nkt - 1))
                        if kt == nkt - 1:
                            evict(dst[:, ch * 512:(ch + 1) * 512],
                                  state['ps'][:], bcol, nc.vector)
                    items.append((213, f, label if kt == nkt - 1 else None))
                return items

            # V projection: channel-major [ch, tok] into a staging tile,
            # then per-(k-tile, head) DMA transposes into the padded vn
            # layout ([ones|pad63|v64] per head).  V bias is added to the
            # normalized ya/yb in the part epilogue (softmax-weighted
            # average of a constant is the constant).
            def v_items(wtile, nkt, xsrc, ch, dstn, ktbase, label):
                items = []
                state = {}
                for kt in range(nkt):
                    def f(kt=kt, state=state):
                        if kt == 0:
                            state['ps'] = auxps.tile([128, 512], f32, tag="aux", name="auxtile")
                        nc.tensor.matmul(state['ps'][:], wtile[:, kt, :],
                                         xsrc[:, kt, ch * 512:(ch + 1) * 512],
                                         start=(kt == 0), stop=(kt == nkt - 1))
                        if kt == nkt - 1:
                            vt = vtpool.tile([128, 512], f16, tag="vt",
                                             name="vttile")
                            nc.vector.tensor_copy(vt[:], state['ps'][:])
                            dr = dstn[:].rearrange(
                                "p (t h c) -> p t h c", h=2, c=128)
                            for tt in range(4):
                                for h in range(2):
                                    nc.sync.dma_start_transpose(
                                        out=dr[:, ktbase + tt, h, 64:128],
                                        in_=vt[h * 64:(h + 1) * 64,
                                               tt * 128:(tt + 1) * 128])
                    items.append((213, f, label if kt == nkt - 1 else None))
                return items

            def c_items(qlo):
                items = []
                for tti in range(4):
                    tt = qlo // 128 + tti
                    state = {}
                    for co in range(2):
                        def f(tt=tt, co=co, state=state):
                            if co == 0:
                                state['so'] = cpool.tile([128, C], f16,
                                                         tag="so", name="sotile")
                            po = auxps.tile([128, 512], f32, tag="aux",
                                            name="auxtile")
                            nc.tensor.matmul(po[:],
                                             yT2[:, tt * 128:(tt + 1) * 128],
                                             wp_t[:, co * 512:(co + 1) * 512],
                                             start=True, stop=True)
                            nc.vector.tensor_copy(
                                state['so'][:, co * 512:(co + 1) * 512], po[:])
                            if co == 1:
                                nc.sync.dma_start(
                                    out=outd[tt * 128:(tt + 1) * 128, :],
                                    in_=state['so'][:])
                        items.append((280, f, None))
                return items

            def add_group(items):
                if items[-1][2] is not None:
                    pending.add(items[-1][2])
                FILL.extend(items)

            # ---------------- prologue: b0/qc0 projections ----------------
            for it in proj_items(wck_t, KT_C, call, 0, kcT, 4, None):
                it[1]()
            for it in v_items(wcv_t, KT_C, call, 0, vcn, 0, None):
                it[1]()
            for it in proj_items(wcq_t, KT_X, xall, 0, qcT, 3, None):
                it[1]()
            for it in proj_items(wk_t, KT_X, xall, 0, kT, 1, None):
                it[1]()
            for it in v_items(wv_t, KT_X, xall, 0, vn, 0, None):
                it[1]()
            for it in proj_items(wq_t, KT_X, xall, 0, qT, 0, None):
                it[1]()

            # ---------------- filler queue: remaining projections ----------
            for ch in range(1, 4):
                add_group(proj_items(wcq_t, KT_X, xall, ch, qcT, 3, f"cq{ch}"))
                add_group(proj_items(wk_t, KT_X, xall, ch, kT, 1, f"k{ch}"))
                add_group(v_items(wv_t, KT_X, xall, ch, vn, ch * 4, f"v{ch}"))
                add_group(proj_items(wq_t, KT_X, xall, ch, qT, 0, f"q{ch}"))
            # cross b1
            add_group(proj_items(wck_t, KT_C, call, 1, kcT, 4, "kc1"))
            add_group(v_items(wcv_t, KT_C, call, 1, vcn, 4, "vc1"))
            for ch in range(4, 8):
                add_group(proj_items(wcq_t, KT_X, xall, ch, qcT, 3, f"cq{ch}"))
                add_group(proj_items(wk_t, KT_X, xall, ch, kT, 1, f"k{ch}"))
                add_group(v_items(wv_t, KT_X, xall, ch, vn, ch * 4, f"v{ch}"))
                add_group(proj_items(wq_t, KT_X, xall, ch, qT, 0, f"q{ch}"))

            # ---------------- attention ----------------
            def attn_part(b, qc, qlo, is_self):
                nkt = (4 * qc + 4) if is_self else KT_C
                yh0 = yh0ps.tile([128, 512], f32, tag="yh0")
                yh1 = yh1ps.tile([128, 512], f32, tag="yh1")
                pend = []
                fidx = [0]

                def flush_one():
                    ex, off, vsrc, vc0, vc1 = pend.pop(0)
                    first = fidx[0] == 0
                    last = fidx[0] == nkt - 1
                    fidx[0] += 1
                    nc.tensor.matmul(yh0[:, off:512], vsrc[:, vc0:vc0 + 128],
                                     ex[:, off:512], start=first, stop=last)
                    nc.tensor.matmul(yh1[:, off:512], vsrc[:, vc1:vc1 + 128],
                                     ex[:, 512 + off:1024], start=first, stop=last)

                for kt in range(nkt):
                    if is_self:
                        crossing = kt >= 4 * qc
                        off = (kt - 4 * qc) * 128 if crossing else 0
                        klo = b * T + kt * 128
                        ksrc, qsrc, vsrc = kT, qT, vn
                        vbase = (b * KT_PER_B + kt) * 256
                    else:
                        crossing, off = False, 0
                        klo = b * TC + kt * 128
                        ksrc, qsrc, vsrc = kcT, qcT, vcn
                        vbase = (b * KT_C + kt) * 256
                    st = stps.tile([128, 1024], f32, tag="st")
                    nc.tensor.matmul(st[:, off:512],
                                     ksrc[0:64, klo:klo + 128],
                                     qsrc[0:64, qlo + off:qlo + 512],
                                     start=True, stop=True)
                    nc.tensor.matmul(st[:, 512 + off:1024],
                                     ksrc[64:128, klo:klo + 128],
                                     qsrc[64:128, qlo + off:qlo + 512],
                                     start=True, stop=True)
                    ex = expool.tile([128, 1024], f16, tag="ex")
                    if off == 0:
                        nc.scalar.activation(ex[:], st[:], Exp, scale=SCALE)
                    else:
                        st3 = st[:].rearrange("p (h q) -> p h q", h=2)[:, :, off:512]
                        ex3 = ex[:].rearrange("p (h q) -> p h q", h=2)[:, :, off:512]
                        nc.scalar.activation(ex3, st3, Exp, scale=SCALE)
                    if crossing:
                        nc.gpsimd.tensor_mul(ex[:, off:off + 128],
                                             ex[:, off:off + 128], mask_t[:])
                        nc.gpsimd.tensor_mul(ex[:, 512 + off:512 + off + 128],
                                             ex[:, 512 + off:512 + off + 128],
                                             mask_t[:])
                    pend.append((ex, off, vsrc, vbase, vbase + 128))
                    if len(pend) > 1:
                        flush_one()
                    pump(430 if b == 0 else 700)
                while pend:
                    flush_one()
                    pump(300)

                # epilogue: y rows scaled by ALPHA to fp16; denominator row
                # (= ALPHA*den, ones column is ALPHA) reciprocated from PSUM
                # in fp32; ya = (ALPHA*y) * 1/(ALPHA*den) = y/den.
                ysb = bpool.tile([64, 1024], f16, tag="ysb")
                nc.vector.tensor_scalar_mul(ysb[:, 0:512], yh0[64:128, :], ALPHA)
                nc.vector.tensor_scalar_mul(ysb[:, 512:1024], yh1[64:128, :], ALPHA)
                rsb = bpool.tile([1, 1024], f32, tag="rsb")
                nc.vector.reciprocal_approx_fast(rsb[:, 0:512], yh0[0:1, :])
                nc.vector.reciprocal_approx_fast(rsb[:, 512:1024], yh1[0:1, :])
                rrd = rdpool.tile([1, 1024], f32, tag="rrd", name="rrdtile")
                nc.sync.dma_start(out=rrd[:], in_=rsb[:])
                # broadcast via GpSimd SWDGE: casts f32->f16 and gives a real
                # cross-engine wait on the rrd write (sync-queue DMA)
                bc = ypool.tile([64, 1024], f16, tag="bc")
                nc.gpsimd.dma_start(out=bc[:],
                                    in_=rrd[0:1, :].to_broadcast((64, 1024)))
                ya = ypool.tile([64, 512], f16, tag="ya")
                nc.gpsimd.tensor_mul(ya[:], ysb[:, 0:512], bc[:, 0:512])
                yb = ypool.tile([64, 512], f16, tag="yb")
                nc.gpsimd.tensor_mul(yb[:], ysb[:, 512:1024], bc[:, 512:1024])
                if not zero_bias:
                    vb = 2 if is_self else 5
                    nc.gpsimd.tensor_scalar_add(ya[:], ya[:],
                                                bias_t[0:64, vb:vb + 1])
                    nc.gpsimd.tensor_scalar_add(yb[:], yb[:],
                                                bias_t[64:128, vb:vb + 1])
                return ya, yb

            for b in range(B):
                for qc in range(QC_PER_B):
                    qlo = b * T + qc * 512
                    chk = b * 4 + qc
                    if b == 1:
                        require("kc1")
                        require("vc1")
                    if chk > 0:
                        require(f"cq{chk}")
                    ya_c, yb_c = attn_part(b, qc, qlo, is_self=False)
                    if chk > 0:
                        require(f"k{chk}")
                        require(f"v{chk}")
                        require(f"q{chk}")
                    ya_s, yb_s = attn_part(b, qc, qlo, is_self=True)
                    nc.gpsimd.tensor_add(yT2[0:64, qlo:qlo + 512],
                                         ya_s[:], ya_c[:])
                    ybsum = ypool.tile([64, 512], f16, tag="ybsum")
                    nc.gpsimd.tensor_add(ybsum[:], yb_s[:], yb_c[:])
                    # partition shift rows 0-63 -> 64-127 via SBUF-SBUF DMA
                    nc.sync.dma_start(out=yT2[64:128, qlo:qlo + 512],
                                      in_=ybsum[:])
                    add_group(c_items(qlo))

            while FILL:
                run_item(FILL.pop(0))

    nc.compile()
    return nc


_NC_CACHE = {}


def _get_nc(zero_bias=False):
    if zero_bias not in _NC_CACHE:
        _NC_CACHE[zero_bias] = _build(zero_bias)
    return _NC_CACHE[zero_bias]


def warr(w):
    """[C,128] weight -> [128, KT, 128] fp16 (partition-major k-tiles)."""
    kt = w.shape[0] // 128
    return np.ascontiguousarray(
        w.reshape(kt, 128, w.shape[1]).transpose(1, 0, 2)).astype(np.float16)


def make_in_maps(x, cross_input, Wk, bk, Wq, bq, Wv, bv, Wck, bck, Wcq, bcq,
                 Wcv, bcv, Wp, bp):
    """Host-side shard + layout prep. Returns per-core input maps."""
    xT0 = np.asarray(x, np.float32).reshape(NT, C).T.astype(np.float16)  # [C, NT]
    xT = np.ascontiguousarray(
        xT0.reshape(KT_X, 128, NCH, 512).transpose(2, 1, 0, 3))  # [NCH,128,KT,512]
    cT0 = np.asarray(cross_input, np.float32).reshape(NTC, CC).T.astype(np.float16)
    cT = np.ascontiguousarray(
        cT0.reshape(KT_C, 128, NCHC, 512).transpose(2, 1, 0, 3))
    mask = np.triu(np.ones((128, 128), np.float32)).astype(np.float16)  # 1 iff kk<=qq
    Wq, Wk, Wv = (np.asarray(w, np.float32) for w in (Wq, Wk, Wv))
    Wcq, Wck, Wcv = (np.asarray(w, np.float32) for w in (Wcq, Wck, Wcv))
    Wp = np.asarray(Wp, np.float32)
    in_maps = []
    for c in range(NCORES):
        sl = slice(c * CPC, (c + 1) * CPC)
        bias6 = np.stack([np.asarray(v, np.float32)[sl] for v in
                          (bq, bk, bv, bcq, bck, bcv)], axis=1)
        in_maps.append({
            "xT": xT, "cT": cT,
            "wq": warr(Wq[:, sl]), "wk": warr(Wk[:, sl]),
            "wv": warr(Wv[:, sl]), "wcq": warr(Wcq[:, sl]),
            "wck": warr(Wck[:, sl]), "wcv": warr(Wcv[:, sl]),
            "wp": Wp[sl, :].astype(np.float16),
            "bias6": np.ascontiguousarray(bias6),
            "mask": mask,
        })
    return in_maps


def kernel(**inputs):
    in_maps = make_in_maps(**inputs)
    zb = all(not np.any(np.asarray(inputs[k])) for k in
             ("bq", "bk", "bv", "bcq", "bck", "bcv"))
    nc = _get_nc(zero_bias=zb)
    res = run_bass_kernel_spmd(nc, in_maps, list(range(NCORES)))
    acc = np.zeros((NT, C), np.float64)
    for c in range(NCORES):
        acc += res.results[c]["out"]
    acc += np.asarray(inputs["bp"], np.float32)
    return acc.reshape(B, T, C).astype(np.float32)


if __name__ == "__main__":
    nc = _get_nc()
    print("build + compile OK")


# revision 25
# speedup vs baseline: 1.3244x; 1.3244x over previous
"""Causal self-attention + cross-attention Trainium2 kernel (8 NeuronCores).

Sharding: head-parallel. 16 heads x 2 batches = 32 (b,h) pairs; core c owns
heads {2c, 2c+1} for both batches (its 128 channels of C=1024). Projections
are column-sliced per core; attention runs fully local per head; the output
projection is row-sliced and the 8 partial [NT, C] fp32 outputs are summed
on the host (no device collectives).

v2: single software-pipelined emission. ScalarE runs ONLY the exp stream
(the kernel's critical resource); projections, the output projection and
V-layout work are interleaved into the PE queue as filler between the
per-k-tile score/AV steps so PE and ScalarE stay busy concurrently from
~8us onward. PSUM->SBUF evictions run on GpSimd/DVE, output tiles DMA
straight from PSUM, V is projected token-major directly (no transposes),
and the softmax epilogue runs in fp16 with a 1/64-scaled ones column.
"""
import sys

sys.path.insert(0, "/opt/trn_rl_repo")

import numpy as np

import concourse.bass as bass
import concourse.tile as tile
from concourse import bacc, mybir
from concourse.bass_utils import run_bass_kernel_spmd

dt = mybir.dt

B, T, TC, C, CC, H, D = 2, 2048, 512, 1024, 512, 16, 64
NCORES = 8
CPC = 128          # channels per core = 2 heads * 64
NT = B * T         # 4096 tokens (batch-major)
NTC = B * TC       # 1024 cross tokens
KT_X = C // 128    # 8 contraction tiles over C
KT_C = CC // 128   # 4 contraction tiles over CC
NCH = NT // 512    # 8 token chunks (b0: 0-3, b1: 4-7)
NCHC = NTC // 512  # 2 cross chunks (b0, b1)
QC_PER_B = T // 512
KT_PER_B = T // 128
ALPHA = 1.0 / 64   # ones-column value; denominators come out scaled by ALPHA


def _build(zero_bias=False):
    f32, f16 = dt.float32, dt.float16
    nc = bacc.Bacc("TRN2", target_bir_lowering=False, debug=False,
                   enable_asserts=True, num_devices=NCORES)

    xTd = nc.dram_tensor("xT", [NCH, 128, KT_X, 512], f16, kind="ExternalInput").ap()
    cTd = nc.dram_tensor("cT", [NCHC, 128, KT_C, 512], f16, kind="ExternalInput").ap()
    wqd = nc.dram_tensor("wq", [128, KT_X, CPC], f16, kind="ExternalInput").ap()
    wkd = nc.dram_tensor("wk", [128, KT_X, CPC], f16, kind="ExternalInput").ap()
    wvd = nc.dram_tensor("wv", [128, KT_X, CPC], f16, kind="ExternalInput").ap()
    wcqd = nc.dram_tensor("wcq", [128, KT_X, CPC], f16, kind="ExternalInput").ap()
    wckd = nc.dram_tensor("wck", [128, KT_C, CPC], f16, kind="ExternalInput").ap()
    wcvd = nc.dram_tensor("wcv", [128, KT_C, CPC], f16, kind="ExternalInput").ap()
    wpd = nc.dram_tensor("wp", [CPC, C], f16, kind="ExternalInput").ap()
    bias6d = nc.dram_tensor("bias6", [CPC, 6], f32, kind="ExternalInput").ap()
    maskd = nc.dram_tensor("mask", [128, 128], f16, kind="ExternalInput").ap()
    outd = nc.dram_tensor("out", [NT, C], f16, kind="ExternalOutput").ap()

    Exp = mybir.ActivationFunctionType.Exp
    Mult = mybir.AluOpType.mult
    SCALE = 0.125  # 1/sqrt(D)

    with tile.TileContext(nc) as tc:
        from contextlib import ExitStack
        with ExitStack() as es:
            persist = es.enter_context(tc.tile_pool(name="persist", bufs=1))
            qT = persist.tile([128, NT], f16, tag="qT")
            kT = persist.tile([128, NT], f16, tag="kT")
            qcT = persist.tile([128, NT], f16, tag="qcT")
            kcT = persist.tile([128, NTC], f16, tag="kcT")
            vn = persist.tile([128, (NT // 128) * 256], f16, tag="vn")
            vcn = persist.tile([128, (NTC // 128) * 256], f16, tag="vcn")
            yT2 = persist.tile([128, NT], f16, tag="yT2")
            wp_t = persist.tile([128, C], f16, tag="wp")
            bias_t = persist.tile([128, 6], f32, tag="bias")
            mask_t = persist.tile([128, 128], f16, tag="mask")
            xall = persist.tile([128, KT_X, NT], f16, tag="xall")
            call = persist.tile([128, KT_C, NTC], f16, tag="call")
            wq_t = persist.tile([128, KT_X, CPC], f16, tag="wqw")
            wk_t = persist.tile([128, KT_X, CPC], f16, tag="wkw")
            wv_t = persist.tile([128, KT_X, CPC], f16, tag="wvw")
            wcq_t = persist.tile([128, KT_X, CPC], f16, tag="wcqw")
            wck_t = persist.tile([128, KT_C, CPC], f16, tag="wckw")
            wcv_t = persist.tile([128, KT_C, CPC], f16, tag="wcvw")

            expool = es.enter_context(tc.tile_pool(name="expool", bufs=6))
            bpool = es.enter_context(tc.tile_pool(name="bpool", bufs=3))
            ypool = es.enter_context(tc.tile_pool(name="ypool", bufs=3))
            cpool = es.enter_context(tc.tile_pool(name="cpool", bufs=3))
            rdpool = es.enter_context(tc.tile_pool(name="rdpool", bufs=3, space="DRAM"))
            stps = es.enter_context(tc.tile_pool(name="stps", bufs=2, space="PSUM"))
            yh0ps = es.enter_context(tc.tile_pool(name="yh0ps", bufs=1, space="PSUM"))
            yh1ps = es.enter_context(tc.tile_pool(name="yh1ps", bufs=1, space="PSUM"))
            auxps = es.enter_context(tc.tile_pool(name="auxps", bufs=2, space="PSUM"))

            # ---------------- input DMAs (SP queue, critical first) ----
            for wdram, wtile in ((wckd, wck_t), (wcvd, wcv_t)):
                nc.sync.dma_start(out=wtile[:], in_=wdram[:])
            nc.sync.dma_start(out=mask_t[:], in_=maskd[:])
            nc.sync.dma_start(out=call[:, :, 0:512], in_=cTd[0])
            nc.sync.dma_start(out=wcq_t[:], in_=wcqd[:])
            nc.sync.dma_start(out=xall[:, :, 0:512], in_=xTd[0])
            for wdram, wtile in ((wkd, wk_t), (wvd, wv_t), (wqd, wq_t)):
                nc.sync.dma_start(out=wtile[:], in_=wdram[:])
            nc.sync.dma_start(out=bias_t[:], in_=bias6d[:])
            for ch in range(1, 4):
                nc.sync.dma_start(out=xall[:, :, ch * 512:(ch + 1) * 512],
                                  in_=xTd[ch])
            nc.sync.dma_start(out=wp_t[:], in_=wpd[:])
            nc.sync.dma_start(out=call[:, :, 512:1024], in_=cTd[1])
            for ch in range(4, 8):
                nc.sync.dma_start(out=xall[:, :, ch * 512:(ch + 1) * 512],
                                  in_=xTd[ch])

            # zero only the pad columns (yh pad rows are never read, but the
            # sim's nnan checks see matmul outputs; keep pads defined)
            vn_h = vn[:].rearrange("p (t h c) -> p t h c", h=2, c=128)
            nc.gpsimd.memset(vn_h[:, :, :, 1:64], 0.0)
            nc.gpsimd.memset(vn_h[:, :, :, 0:1], ALPHA)
            vcn_h = vcn[:].rearrange("p (t h c) -> p t h c", h=2, c=128)
            nc.gpsimd.memset(vcn_h[:, :, :, 1:64], 0.0)
            nc.gpsimd.memset(vcn_h[:, :, :, 0:1], ALPHA)

            # ---------------- filler machinery ----------------
            FILL = []            # list of (ns_est, fn, label_or_None)
            pending = set()

            def run_item(item):
                _, fn, label = item
                fn()
                if label is not None:
                    pending.discard(label)

            def pump(budget_ns):
                while budget_ns > 0 and FILL:
                    item = FILL.pop(0)
                    run_item(item)
                    budget_ns -= item[0]

            def require(label):
                while label in pending:
                    run_item(FILL.pop(0))

            def evict(dst, ps, bcol, eng):
                if zero_bias:
                    eng.tensor_copy(dst, ps)
                else:
                    eng.tensor_scalar_add(dst, ps, bias_t[:, bcol:bcol + 1])

            # channel-major projection group: dst[:, ch*512:+512] = W^T x_chunk
            def proj_items(wtile, nkt, xsrc, ch, dst, bcol, label):
                items = []
                state = {}
                for kt in range(nkt):
                    def f(kt=kt, state=state):
                        if kt == 0:
                            state['ps'] = auxps.tile([128, 512], f32, tag="aux", name="auxtile")
                        nc.tensor.matmul(state['ps'][:], wtile[:, kt, :],
                                         xsrc[:, kt, ch * 512:(ch + 1) * 512],
                                         start=(kt == 0), stop=(kt == nkt - 1))
                        if kt == nkt - 1:
                            evict(dst[:, ch * 512:(ch + 1) * 512],
                                  state['ps'][:], bcol, nc.vector)
                    items.append((213, f, label if kt == nkt - 1 else None))
                return items

            # token-major V projection, one 128-token tile per group:
            # out[tok, ch] = sum_kt xallT[ckt, tok]^T @ Wv[ckt, ch], written
            # into the padded vn layout ([ones|pad63|v64] per head).  V bias
            # is added to the normalized ya/yb in the part epilogue
            # (softmax-weighted average of a constant is the constant).
            def v_items(wtile, nkt, xsrc, tglob, dstn, dtile, label):
                items = []
                state = {}
                for kt in range(nkt):
                    def f(kt=kt, state=state):
                        if kt == 0:
                            state['ps'] = auxps.tile([128, 512], f32, tag="aux", name="auxtile")
                        nc.tensor.matmul(state['ps'][:, 0:128],
                                         xsrc[:, kt, tglob * 128:(tglob + 1) * 128],
                                         wtile[:, kt, :],
                                         start=(kt == 0), stop=(kt == nkt - 1))
                        if kt == nkt - 1:
                            ps = state['ps']
                            dr = dstn[:].rearrange(
                                "p (t h c) -> p t h c", h=2, c=128)
                            nc.vector.tensor_copy(
                                dr[:, dtile, :, 64:128],
                                ps[:, 0:128].rearrange("p (h c) -> p h c", h=2))
                    items.append((140, f, label if kt == nkt - 1 else None))
                return items

            def c_items(qlo):
                items = []
                for tti in range(4):
                    tt = qlo // 128 + tti
                    state = {}
                    for co in range(2):
                        def f(tt=tt, co=co, state=state):
                            if co == 0:
                                state['so'] = cpool.tile([128, C], f16,
                                                         tag="so", name="sotile")
                            po = auxps.tile([128, 512], f32, tag="aux",
                                            name="auxtile")
                            nc.tensor.matmul(po[:],
                                             yT2[:, tt * 128:(tt + 1) * 128],
                                             wp_t[:, co * 512:(co + 1) * 512],
                                             start=True, stop=True)
                            nc.vector.tensor_copy(
                                state['so'][:, co * 512:(co + 1) * 512], po[:])
                            if co == 1:
                                nc.sync.dma_start(
                                    out=outd[tt * 128:(tt + 1) * 128, :],
                                    in_=state['so'][:])
                        items.append((280, f, None))
                return items

            def add_group(items):
                if items[-1][2] is not None:
                    pending.add(items[-1][2])
                FILL.extend(items)

            # ---------------- prologue: b0/qc0 projections ----------------
            for it in proj_items(wck_t, KT_C, call, 0, kcT, 4, None):
                it[1]()
            for tti in range(4):
                for it in v_items(wcv_t, KT_C, call, tti, vcn, tti, None):
                    it[1]()
            for it in proj_items(wcq_t, KT_X, xall, 0, qcT, 3, None):
                it[1]()
            for it in proj_items(wk_t, KT_X, xall, 0, kT, 1, None):
                it[1]()
            for tti in range(4):
                for it in v_items(wv_t, KT_X, xall, tti, vn, tti, None):
                    it[1]()
            for it in proj_items(wq_t, KT_X, xall, 0, qT, 0, None):
                it[1]()

            # ---------------- filler queue: remaining projections ----------
            for ch in range(1, 4):
                add_group(proj_items(wcq_t, KT_X, xall, ch, qcT, 3, f"cq{ch}"))
                add_group(proj_items(wk_t, KT_X, xall, ch, kT, 1, f"k{ch}"))
                vi = []
                for tti in range(4):
                    vi += v_items(wv_t, KT_X, xall, ch * 4 + tti, vn,
                                  ch * 4 + tti, f"v{ch}" if tti == 3 else None)
                add_group(vi)
                add_group(proj_items(wq_t, KT_X, xall, ch, qT, 0, f"q{ch}"))
            # cross b1
            add_group(proj_items(wck_t, KT_C, call, 1, kcT, 4, "kc1"))
            vi = []
            for tti in range(4):
                vi += v_items(wcv_t, KT_C, call, 4 + tti, vcn, 4 + tti,
                              "vc1" if tti == 3 else None)
            add_group(vi)
            for ch in range(4, 8):
                add_group(proj_items(wcq_t, KT_X, xall, ch, qcT, 3, f"cq{ch}"))
                add_group(proj_items(wk_t, KT_X, xall, ch, kT, 1, f"k{ch}"))
                vi = []
                for tti in range(4):
                    vi += v_items(wv_t, KT_X, xall, ch * 4 + tti, vn,
                                  ch * 4 + tti, f"v{ch}" if tti == 3 else None)
                add_group(vi)
                add_group(proj_items(wq_t, KT_X, xall, ch, qT, 0, f"q{ch}"))

            # ---------------- attention ----------------
            def attn_part(b, qc, qlo, is_self):
                nkt = (4 * qc + 4) if is_self else KT_C
                yh0 = yh0ps.tile([128, 512], f32, tag="yh0")
                yh1 = yh1ps.tile([128, 512], f32, tag="yh1")
                pend = []
                fidx = [0]

                def flush_one():
                    ex, off, vsrc, vc0, vc1 = pend.pop(0)
                    first = fidx[0] == 0
                    last = fidx[0] == nkt - 1
                    fidx[0] += 1
                    nc.tensor.matmul(yh0[:, off:512], vsrc[:, vc0:vc0 + 128],
                                     ex[:, off:512], start=first, stop=last)
                    nc.tensor.matmul(yh1[:, off:512], vsrc[:, vc1:vc1 + 128],
                                     ex[:, 512 + off:1024], start=first, stop=last)

                for kt in range(nkt):
                    if is_self:
                        crossing = kt >= 4 * qc
                        off = (kt - 4 * qc) * 128 if crossing else 0
                        klo = b * T + kt * 128
                        ksrc, qsrc, vsrc = kT, qT, vn
                        vbase = (b * KT_PER_B + kt) * 256
                    else:
                        crossing, off = False, 0
                        klo = b * TC + kt * 128
                        ksrc, qsrc, vsrc = kcT, qcT, vcn
                        vbase = (b * KT_C + kt) * 256
                    st = stps.tile([128, 1024], f32, tag="st")
                    nc.tensor.matmul(st[:, off:512],
                                     ksrc[0:64, klo:klo + 128],
                                     qsrc[0:64, qlo + off:qlo + 512],
                                     start=True, stop=True)
                    nc.tensor.matmul(st[:, 512 + off:1024],
                                     ksrc[64:128, klo:klo + 128],
                                     qsrc[64:128, qlo + off:qlo + 512],
                                     start=True, stop=True)
                    ex = expool.tile([128, 1024], f16, tag="ex")
                    if off == 0:
                        nc.scalar.activation(ex[:], st[:], Exp, scale=SCALE)
                    else:
                        st3 = st[:].rearrange("p (h q) -> p h q", h=2)[:, :, off:512]
                        ex3 = ex[:].rearrange("p (h q) -> p h q", h=2)[:, :, off:512]
                        nc.scalar.activation(ex3, st3, Exp, scale=SCALE)
                    if crossing:
                        nc.gpsimd.tensor_mul(ex[:, off:off + 128],
                                             ex[:, off:off + 128], mask_t[:])
                        nc.gpsimd.tensor_mul(ex[:, 512 + off:512 + off + 128],
                                             ex[:, 512 + off:512 + off + 128],
                                             mask_t[:])
                    pend.append((ex, off, vsrc, vbase, vbase + 128))
                    if len(pend) > 1:
                        flush_one()
                    pump(430 if b == 0 else 700)
                while pend:
                    flush_one()
                    pump(300)

                # epilogue: y rows scaled by ALPHA to fp16; denominator row
                # (= ALPHA*den, ones column is ALPHA) reciprocated from PSUM
                # in fp32; ya = (ALPHA*y) * 1/(ALPHA*den) = y/den.
                ysb = bpool.tile([64, 1024], f16, tag="ysb")
                nc.vector.tensor_scalar_mul(ysb[:, 0:512], yh0[64:128, :], ALPHA)
                nc.vector.tensor_scalar_mul(ysb[:, 512:1024], yh1[64:128, :], ALPHA)
                rsb = bpool.tile([1, 1024], f32, tag="rsb")
                nc.vector.reciprocal_approx_fast(rsb[:, 0:512], yh0[0:1, :])
                nc.vector.reciprocal_approx_fast(rsb[:, 512:1024], yh1[0:1, :])
                rrd = rdpool.tile([1, 1024], f32, tag="rrd", name="rrdtile")
                nc.sync.dma_start(out=rrd[:], in_=rsb[:])
                # broadcast via GpSimd SWDGE: casts f32->f16 and gives a real
                # cross-engine wait on the rrd write (sync-queue DMA)
                bc = ypool.tile([64, 1024], f16, tag="bc")
                nc.gpsimd.dma_start(out=bc[:],
                                    in_=rrd[0:1, :].to_broadcast((64, 1024)))
                ya = ypool.tile([64, 512], f16, tag="ya")
                nc.gpsimd.tensor_mul(ya[:], ysb[:, 0:512], bc[:, 0:512])
                yb = ypool.tile([64, 512], f16, tag="yb")
                nc.gpsimd.tensor_mul(yb[:], ysb[:, 512:1024], bc[:, 512:1024])
                if not zero_bias:
                    vb = 2 if is_self else 5
                    nc.gpsimd.tensor_scalar_add(ya[:], ya[:],
                                                bias_t[0:64, vb:vb + 1])
                    nc.gpsimd.tensor_scalar_add(yb[:], yb[:],
                                                bias_t[64:128, vb:vb + 1])
                return ya, yb

            for b in range(B):
                for qc in range(QC_PER_B):
                    qlo = b * T + qc * 512
                    chk = b * 4 + qc
                    if b == 1:
                        require("kc1")
                        require("vc1")
                    if chk > 0:
                        require(f"cq{chk}")
                    ya_c, yb_c = attn_part(b, qc, qlo, is_self=False)
                    if chk > 0:
                        require(f"k{chk}")
                        require(f"v{chk}")
                        require(f"q{chk}")
                    ya_s, yb_s = attn_part(b, qc, qlo, is_self=True)
                    nc.gpsimd.tensor_add(yT2[0:64, qlo:qlo + 512],
                                         ya_s[:], ya_c[:])
                    ybsum = ypool.tile([64, 512], f16, tag="ybsum")
                    nc.gpsimd.tensor_add(ybsum[:], yb_s[:], yb_c[:])
                    # partition shift rows 0-63 -> 64-127 via SBUF-SBUF DMA
                    nc.sync.dma_start(out=yT2[64:128, qlo:qlo + 512],
                                      in_=ybsum[:])
                    add_group(c_items(qlo))

            while FILL:
                run_item(FILL.pop(0))

    nc.compile()
    return nc


_NC_CACHE = {}


def _get_nc(zero_bias=False):
    if zero_bias not in _NC_CACHE:
        _NC_CACHE[zero_bias] = _build(zero_bias)
    return _NC_CACHE[zero_bias]


def warr(w):
    """[C,128] weight -> [128, KT, 128] fp16 (partition-major k-tiles)."""
    kt = w.shape[0] // 128
    return np.ascontiguousarray(
        w.reshape(kt, 128, w.shape[1]).transpose(1, 0, 2)).astype(np.float16)


def make_in_maps(x, cross_input, Wk, bk, Wq, bq, Wv, bv, Wck, bck, Wcq, bcq,
                 Wcv, bcv, Wp, bp):
    """Host-side shard + layout prep. Returns per-core input maps."""
    xT0 = np.asarray(x, np.float32).reshape(NT, C).T.astype(np.float16)  # [C, NT]
    xT = np.ascontiguousarray(
        xT0.reshape(KT_X, 128, NCH, 512).transpose(2, 1, 0, 3))  # [NCH,128,KT,512]
    cT0 = np.asarray(cross_input, np.float32).reshape(NTC, CC).T.astype(np.float16)
    cT = np.ascontiguousarray(
        cT0.reshape(KT_C, 128, NCHC, 512).transpose(2, 1, 0, 3))
    mask = np.triu(np.ones((128, 128), np.float32)).astype(np.float16)  # 1 iff kk<=qq
    Wq, Wk, Wv = (np.asarray(w, np.float32) for w in (Wq, Wk, Wv))
    Wcq, Wck, Wcv = (np.asarray(w, np.float32) for w in (Wcq, Wck, Wcv))
    Wp = np.asarray(Wp, np.float32)
    in_maps = []
    for c in range(NCORES):
        sl = slice(c * CPC, (c + 1) * CPC)
        bias6 = np.stack([np.asarray(v, np.float32)[sl] for v in
                          (bq, bk, bv, bcq, bck, bcv)], axis=1)
        in_maps.append({
            "xT": xT, "cT": cT,
            "wq": warr(Wq[:, sl]), "wk": warr(Wk[:, sl]),
            "wv": warr(Wv[:, sl]), "wcq": warr(Wcq[:, sl]),
            "wck": warr(Wck[:, sl]), "wcv": warr(Wcv[:, sl]),
            "wp": Wp[sl, :].astype(np.float16),
            "bias6": np.ascontiguousarray(bias6),
            "mask": mask,
        })
    return in_maps


def kernel(**inputs):
    in_maps = make_in_maps(**inputs)
    zb = all(not np.any(np.asarray(inputs[k])) for k in
             ("bq", "bk", "bv", "bcq", "bck", "bcv"))
    nc = _get_nc(zero_bias=zb)
    res = run_bass_kernel_spmd(nc, in_maps, list(range(NCORES)))
    acc = np.zeros((NT, C), np.float64)
    for c in range(NCORES):
        acc += res.results[c]["out"]
    acc += np.asarray(inputs["bp"], np.float32)
    return acc.reshape(B, T, C).astype(np.float32)


if __name__ == "__main__":
    nc = _get_nc()
    print("build + compile OK")
